# revision 13
# baseline (speedup 1.0000x reference)
"""Trainium2 Bass kernel for nn_AGCRNCellWithMLP (AGCRN cell with per-node MLP weights).

Math (with nodes_ind == arange(N), which the harness guarantees):
    xh       = concat([x, h], -1)                      # [N, 129]
    combined = adj @ xh                                # [N, 129]
    r = sigmoid(mlp(combined, q, W_r, b_r))            # [N, 64]
    u = sigmoid(mlp(combined, q, W_u, b_u))
    h2 = r * h
    cand = tanh(mlp(concat([x, h2], -1), q, W_c, b_c))
    out = (1 - u) * h2 + u * cand
where mlp(v, q, W, b)[n, o] = sum_{d,i} q[n,d] v[n,i] W[d,i,o] + (q @ b)[n, o].

Sharding: data-parallel over nodes, 512 rows per core x 8 cores, no
collectives. All matmul operands fp16 (rel err ~3e-3 vs 2e-2 gate), PSUM fp32,
output fp16.

v2 structure (node-block pipelined, DMA-minimized):
  - DMA per core ~6.3MB (adj 4MB + xh 1.06MB + wdru 0.5 + wdc 0.25 + blobs
    ~0.2): the old 2MB qbp broadcast is generated ON-CHIP by 32 GPSIMD
    partition_broadcast calls ([128,256] per (block, d)); blobA zero-padding
    dropped by packing 16-row and 64-row blobs densely; wdc not duplicated.
  - 512 nodes split into 2 blocks of 256: adjT streamed block-major, so
    block-0 gates (DVE z-muls + PE gate matmuls) run while block-1 adj
    still streams.  Per-block out DMA shortens the tail.
  - Small blobs issue on the scalar HWDGE queue; bulk stream stays ordered
    on sync.  Warm matmuls at start fight the PE p-state ramp.
"""
import sys

sys.path.insert(0, "/opt/trn_rl_repo")

import numpy as np

N = 4096
IN = 64
QD = 16
CI = 2 * IN + 1          # 129
NCORES = 8
NS = N // NCORES         # 512 nodes per core
KT = N // 128            # 32 k-tiles for the adj matmul
CI2 = CI + 1             # xh slab width: 129 + pad col
NB = 2                   # node blocks per core
NSB = NS // NB           # 256 nodes per block
XH0 = KT * CI2           # 4160: cols of xh region in strm
SW = XH0 + NB * KT * NSB  # strm width 20544

# qblob column offsets (fp16, [16, QBW])
QO = {"qT": 0, "bru": 512, "bc": 640, "w128ru": 704, "w128c": 832,
      "x64rep": 896}
QBW = 1408

_CACHE = {}


def build_nc(debug=False):
    import concourse.bass as bass
    import concourse.bacc as bacc
    import concourse.tile as tile
    import concourse.mybir as mybir

    F32 = mybir.dt.float32
    F16 = mybir.dt.float16
    ACT = mybir.ActivationFunctionType

    nc = bacc.Bacc()
    dp = nc.declare_dram_parameter
    qblob_e = dp("qblob", [QD, QBW], F16, isOutput=False)
    qrow_e = dp("qrow", [1, QD * NS], F16, isOutput=False)
    xhblob_e = dp("xhblob", [IN, 2 * NS], F16, isOutput=False)
    strm_e = dp("strm", [128, SW], F16, isOutput=False)
    wdru_e = dp("wdru", [128, QD * 2 * IN], F16, isOutput=False)
    wdc_e = dp("wdc", [128, QD * IN], F16, isOutput=False)
    out_e = dp("out", [IN, NS], F16, isOutput=True)

    def bs(b):
        return slice(b * NSB, (b + 1) * NSB)

    with tile.TileContext(nc) as tc:
        with tc.tile_pool(name="const", bufs=1) as cpool, \
             tc.tile_pool(name="big", bufs=1) as bigpool, \
             tc.tile_pool(name="work", bufs=1) as wpool, \
             tc.tile_pool(name="zt", bufs=3) as ztpool, \
             tc.tile_pool(name="psM", bufs=1, space="PSUM") as psM:

            # ---- DMAs: small blobs on scalar queue, bulk on sync -----------
            qblob = cpool.tile([QD, QBW], F16, tag="qblob")
            nc.scalar.dma_start(qblob[:], qblob_e[:])
            qrow = cpool.tile([1, QD * NS], F16, tag="qrow")
            nc.scalar.dma_start(qrow[:], qrow_e[:])
            xhblob = cpool.tile([IN, 2 * NS], F16, tag="xhblob")
            nc.scalar.dma_start(xhblob[:], xhblob_e[:])

            strm = bigpool.tile([128, SW], F16)
            nc.sync.dma_start(strm[:, 0:XH0], strm_e[:, 0:XH0])
            CH = KT * NSB // 2      # 4096 cols = 16 k-tiles per chunk
            wdru = cpool.tile([128, QD * 2 * IN], F16, tag="wdru")
            wdc = cpool.tile([128, QD * IN], F16, tag="wdc")
            for b in range(NB):
                base = XH0 + b * KT * NSB
                nc.sync.dma_start(strm[:, base:base + CH],
                                  strm_e[:, base:base + CH])
                nc.sync.dma_start(strm[:, base + CH:base + 2 * CH],
                                  strm_e[:, base + CH:base + 2 * CH])
                if b == 0:
                    nc.sync.dma_start(wdru[:], wdru_e[:])
            nc.sync.dma_start(wdc[:], wdc_e[:])

            def qslice(name, w):
                o = QO[name]
                return qblob[0:QD, o:o + w]

            qT = qslice("qT", NS)
            xT = xhblob[0:IN, 0:NS]
            hT = xhblob[0:IN, NS:2 * NS]

            def xh_t(t, a, b2):
                return strm[:, t * CI2 + a: t * CI2 + b2]

            def adj_bt(b, t):
                base = XH0 + b * KT * NSB + t * NSB
                return strm[:, base:base + NSB]

            # sel17b: rows {0,32,64,96} = 1 (pl quad-row reduction matrix)
            sel17b = cpool.tile([128, QD], F16, tag="sel17b")
            nc.vector.memset(sel17b[:], 0.0)
            for j in range(4):
                nc.vector.memset(sel17b[32 * j:32 * j + 1, :], 1.0)

            # ---- warmup: dummy matmuls on scrap keep/get the PE hot --------
            scrap = cpool.tile([128, NS], F16, tag="scrap")
            nc.vector.memset(scrap[:], 0.0)
            ps_scrap = psM.tile([QD, NS], F32, tag="scrapps")

            def warm(n):
                for _ in range(n):
                    nc.tensor.matmul(ps_scrap[:], scrap[0:QD, 0:QD],
                                     scrap[0:QD, :], start=True, stop=True,
                                     skip_group_check=True)

            warm(6)

            # preload sigmoid/tanh activation tables off the critical path
            scr_act = wpool.tile([1, 2], F16, tag="scr_act")
            nc.scalar.activation(scr_act[:], scrap[0:1, 0:2], ACT.Sigmoid)
            scr_act2 = wpool.tile([1, 2], F16, tag="scr_act2")
            nc.scalar.activation(scr_act2[:], scrap[0:1, 0:2], ACT.Tanh)

            # xh2T = [h2|x]: x copied into rows 64:128 on ScalarE
            xh2T = wpool.tile([128, NS], F16, tag="xh2T")
            nc.scalar.copy(xh2T[64:128, :], xT)
            # s_c = q (.) x64, needs only inputs
            s_c = wpool.tile([QD, NS], F16, tag="s_c")
            nc.vector.tensor_mul(s_c[:], qT, qslice("x64rep", NS))

            # ---- qbc on GPSIMD: block-major broadcasts of q rows -----------
            qbp = cpool.tile([128, NB * QD * NSB], F16, tag="qbp")

            def qb_off(b, d):
                return (b * QD + d) * NSB

            for b in range(NB):
                for d in range(QD):
                    o = qb_off(b, d)
                    nc.gpsimd.partition_broadcast(
                        qbp[:, o:o + NSB],
                        qrow[0:1, d * NS + b * NSB: d * NS + (b + 1) * NSB])

            # ---- psum accumulators ----------------------------------------
            gru = psM.tile([128, NS], F32, tag="gru")
            gc = psM.tile([IN, NS], F32, tag="gc")
            pc = psM.tile([128, NS], F32, tag="pc")
            pl = psM.tile([128, NS], F32, tag="pl")
            # quads write only rows {0,32,64,96}; zero the rest so the
            # pl_sb full-tile copy never reads garbage
            nc.vector.memset(pl[:], 0.0)
            v128 = psM.tile([QD, NS], F32, tag="v128")

            # bias matmuls open the PSUM accumulations (one block at a time:
            # a start=True on a sibling slice of the same PSUM bank clobbers
            # the open group, so open block b's group only after block b-1's
            # group on that bank closed)
            def bias_ru(b):
                nc.tensor.matmul(gru[:, bs(b)], qslice("bru", 2 * IN),
                                 qblob[0:QD, bs(b)],
                                 start=True, stop=False, skip_group_check=True)

            def bias_c(b):
                nc.tensor.matmul(gc[:, bs(b)], qslice("bc", IN),
                                 qblob[0:QD, bs(b)],
                                 start=True, stop=False, skip_group_check=True)
                nc.tensor.matmul(gc[:, bs(b)], qslice("w128c", IN),
                                 s_c[:, bs(b)],
                                 start=False, stop=False, skip_group_check=True)

            bias_ru(0)
            bias_c(0)
            warm(2)

            # work tiles (full width, block slices)
            combT = wpool.tile([128, NS], F16, tag="combT")
            pl_sb = wpool.tile([128, NS], F16, tag="pl_sb")
            s_ru = wpool.tile([QD, NS], F16, tag="s_ru")
            r_sb = wpool.tile([IN, NS], F16, tag="r_sb")
            u_sb = wpool.tile([IN, NS], F16, tag="u_sb")
            up_sb = wpool.tile([IN, NS], F16, tag="up_sb")
            cand = wpool.tile([IN, NS], F16, tag="cand")
            e1 = wpool.tile([IN, NS], F16, tag="e1")
            e2 = wpool.tile([IN, NS], F16, tag="e2")
            outT = wpool.tile([IN, NS], F16, tag="outT")

            def adj_phase(b):
                for g in range(8):
                    for t in range(4 * g, 4 * g + 4):
                        nc.tensor.matmul(pc[:, bs(b)], xh_t(t, 0, 128),
                                         adj_bt(b, t),
                                         start=(t == 0), stop=(t == KT - 1),
                                         skip_group_check=True)
                    for t in range(4 * g, 4 * g + 4):
                        j = t % 4
                        nc.tensor.matmul(pl[32 * j:32 * j + 1, bs(b)],
                                         xh_t(t, 128, 129), adj_bt(b, t),
                                         start=(g == 0), stop=(g == 7),
                                         tile_position=(0, 32 * j),
                                         skip_group_check=True)
                nc.scalar.copy(combT[:, bs(b)], pc[:, bs(b)])
                nc.scalar.copy(pl_sb[:, bs(b)], pl[:, bs(b)])
                nc.tensor.matmul(v128[0:QD, bs(b)], sel17b[:], pl_sb[:, bs(b)],
                                 start=True, stop=True, skip_group_check=True)

            def z_quads(b, V, w, wout, acc, name, stop_last=False):
                """4 z-quads [128,4,NSB] = V (.) qbc, each feeding 4 gate
                matmuls accumulating into acc[:, bs(b)]."""
                V4 = V.unsqueeze(1).broadcast_to([128, 4, NSB])
                for jq in range(4):
                    z4 = ztpool.tile([128, 4 * NSB], F16, tag="z",
                                     name=f"z{name}")
                    o = qb_off(b, 4 * jq)
                    nc.vector.tensor_mul(
                        z4[:].rearrange("p (four n) -> p four n", four=4),
                        V4,
                        qbp[:, o:o + 4 * NSB].rearrange(
                            "p (four n) -> p four n", four=4))
                    for k in range(4):
                        d = 4 * jq + k
                        nc.tensor.matmul(acc[:, bs(b)],
                                         w[:, d * wout:(d + 1) * wout],
                                         z4[:, k * NSB:(k + 1) * NSB],
                                         start=False,
                                         stop=(stop_last and d == QD - 1),
                                         skip_group_check=True)

            def ru_gates(b):
                z_quads(b, combT[:, bs(b)], wdru[:], 2 * IN, gru, f"ru{b}")
                nc.vector.tensor_mul(s_ru[:, bs(b)], qblob[0:QD, bs(b)],
                                     v128[0:QD, bs(b)])
                nc.tensor.matmul(gru[:, bs(b)], qslice("w128ru", 2 * IN),
                                 s_ru[:, bs(b)],
                                 start=False, stop=True, skip_group_check=True)
                nc.scalar.activation(r_sb[:, bs(b)], gru[0:IN, bs(b)],
                                     ACT.Sigmoid)
                nc.scalar.activation(u_sb[:, bs(b)], gru[IN:2 * IN, bs(b)],
                                     ACT.Sigmoid)
                nc.scalar.activation(up_sb[:, bs(b)], gru[IN:2 * IN, bs(b)],
                                     ACT.Sigmoid, scale=-1.0)

            def c_gates(b):
                # h2 = r*h -> xh2T rows 0:64; e2 = (1-u)*h2 early
                nc.vector.tensor_mul(xh2T[0:IN, bs(b)], r_sb[:, bs(b)],
                                     hT[:, bs(b)])
                nc.vector.tensor_mul(e2[:, bs(b)], up_sb[:, bs(b)],
                                     xh2T[0:IN, bs(b)])
                z_quads(b, xh2T[:, bs(b)], wdc[:], IN, gc, f"c{b}",
                        stop_last=True)
                nc.scalar.activation(cand[:, bs(b)], gc[0:IN, bs(b)], ACT.Tanh)

            def out_phase(b):
                nc.vector.tensor_mul(e1[:, bs(b)], u_sb[:, bs(b)],
                                     cand[:, bs(b)])
                nc.vector.tensor_add(outT[:, bs(b)], e1[:, bs(b)],
                                     e2[:, bs(b)])
                nc.sync.dma_start(out_e[:, bs(b)], outT[:, bs(b)])

            # gate-c accumulation closes on its last d-matmul: mark stop by
            # emitting c_gates' last matmul with stop=True -> handled below.
            adj_phase(0)
            ru_gates(0)
            bias_ru(1)          # gru-b0 group closed; open b1's
            adj_phase(1)
            c_gates(0)
            bias_c(1)           # gc-b0 group closed; open b1's
            ru_gates(1)
            out_phase(0)
            c_gates(1)
            out_phase(1)

            if debug:
                dbg_tiles = {"combT": combT, "r_sb": r_sb, "u_sb": u_sb,
                             "cand": cand, "xh2T": xh2T, "s_ru": s_ru,
                             "s_c": s_c, "pl_sb": pl_sb, "qbp": qbp,
                             "e1": e1, "e2": e2}
                for nm, t in dbg_tiles.items():
                    shp = list(t[:].shape)
                    de = dp(f"dbg_{nm}", shp, F16, isOutput=True)
                    nc.sync.dma_start(de[:], t[:])
    nc.compile()
    return nc


def _f16(a):
    return np.ascontiguousarray(np.asarray(a, np.float16))


def prep_in_maps(x, h, query_vectors, adj, nodes_ind, W_r, b_r, W_u, b_u, W_c, b_c):
    x = np.asarray(x, np.float32)
    h = np.asarray(h, np.float32)
    q = np.asarray(query_vectors, np.float32)
    adj = np.asarray(adj, np.float32)
    ni = np.asarray(nodes_ind)
    assert np.array_equal(ni, np.arange(N)), "kernel assumes nodes_ind == arange(N)"

    xh = np.concatenate([x, h, np.zeros((N, 1), np.float32)], axis=-1)  # [N,130]
    xh_kt = xh.reshape(KT, 128, CI2).transpose(1, 0, 2)     # [128, KT, 130]

    Wr = np.asarray(W_r, np.float32)
    Wu = np.asarray(W_u, np.float32)
    Wc = np.asarray(W_c, np.float32)
    wdru = np.concatenate([Wr[:, :128, :], Wu[:, :128, :]], axis=2)  # [16,128,128]
    wdru = _f16(wdru.transpose(1, 0, 2).reshape(128, QD * 2 * IN))
    perm_c = list(range(65, CI)) + list(range(0, 64))                # [h2|x]
    wdc = _f16(Wc[:, perm_c, :].transpose(1, 0, 2).reshape(128, QD * IN))

    in_maps = []
    for c in range(NCORES):
        s = slice(c * NS, (c + 1) * NS)
        qs = q[s].T                                             # [16, 512]

        qblob = np.zeros((QD, QBW), np.float32)

        def put(name, arr):
            o = QO[name]
            qblob[0:arr.shape[0], o:o + arr.shape[1]] = arr

        put("qT", qs)
        put("bru", np.concatenate([np.asarray(b_r, np.float32),
                                   np.asarray(b_u, np.float32)], axis=1))
        put("bc", np.asarray(b_c, np.float32))
        put("w128ru", np.concatenate([Wr[:, 128, :], Wu[:, 128, :]], axis=1))
        put("w128c", Wc[:, 64, :])
        put("x64rep", np.tile(x[s, 64], (QD, 1)))

        xhblob = np.concatenate([x[s, 0:64].T, h[s].T], axis=1)  # [64, 1024]

        adjT_kt = adj[s].T.reshape(KT, 128, NS).transpose(1, 0, 2)  # [128,KT,NS]
        strm = np.concatenate(
            [xh_kt.reshape(128, KT * CI2)]
            + [np.ascontiguousarray(adjT_kt[:, :, b * NSB:(b + 1) * NSB])
               .reshape(128, KT * NSB) for b in range(NB)],
            axis=1)
        strm = _f16(strm)

        in_maps.append({
            "qblob": _f16(qblob),
            "qrow": _f16(qs.reshape(1, QD * NS)),
            "xhblob": _f16(xhblob),
            "strm": strm,
            "wdru": wdru, "wdc": wdc,
        })
    return in_maps


def kernel(**inputs):
    from concourse.bass_utils import run_bass_kernel_spmd

    if "nc" not in _CACHE:
        _CACHE["nc"] = build_nc()
    nc = _CACHE["nc"]
    in_maps = prep_in_maps(**inputs)
    res = run_bass_kernel_spmd(nc, in_maps, core_ids=list(range(NCORES)))
    out = np.empty((N, IN), np.float32)
    for c in range(NCORES):
        out[c * NS:(c + 1) * NS, :] = res.results[c]["out"].T.astype(np.float32)
    return out


# revision 17
# speedup vs baseline: 1.2079x; 1.2079x over previous
"""Trainium2 Bass kernel for nn_AGCRNCellWithMLP (AGCRN cell with per-node MLP weights).

Math (with nodes_ind == arange(N), which the harness guarantees):
    xh       = concat([x, h], -1)                      # [N, 129]
    combined = adj @ xh                                # [N, 129]
    r = sigmoid(mlp(combined, q, W_r, b_r))            # [N, 64]
    u = sigmoid(mlp(combined, q, W_u, b_u))
    h2 = r * h
    cand = tanh(mlp(concat([x, h2], -1), q, W_c, b_c))
    out = (1 - u) * h2 + u * cand
where mlp(v, q, W, b)[n, o] = sum_{d,i} q[n,d] v[n,i] W[d,i,o] + (q @ b)[n, o].

Sharding: data-parallel over nodes, 512 rows per core x 8 cores, no
collectives. All matmul operands fp16 (rel err ~3e-3 vs 2e-2 gate), PSUM fp32,
output fp16.

v2 structure (node-block pipelined, DMA-minimized):
  - DMA per core ~6.3MB (adj 4MB + xh 1.06MB + wdru 0.5 + wdc 0.25 + blobs
    ~0.2): the old 2MB qbp broadcast is generated ON-CHIP by 32 GPSIMD
    partition_broadcast calls ([128,256] per (block, d)); blobA zero-padding
    dropped by packing 16-row and 64-row blobs densely; wdc not duplicated.
  - 512 nodes split into 2 blocks of 256: adjT streamed block-major, so
    block-0 gates (DVE z-muls + PE gate matmuls) run while block-1 adj
    still streams.  Per-block out DMA shortens the tail.
  - Small blobs issue on the scalar HWDGE queue; bulk stream stays ordered
    on sync.  Warm matmuls at start fight the PE p-state ramp.
"""
import sys

sys.path.insert(0, "/opt/trn_rl_repo")

import numpy as np

N = 4096
IN = 64
QD = 16
CI = 2 * IN + 1          # 129
NCORES = 8
NS = N // NCORES         # 512 nodes per core
KT = N // 128            # 32 k-tiles for the adj matmul
CI2 = CI + 1             # xh slab width: 129 + pad col
NB = 2                   # node blocks per core
NSB = NS // NB           # 256 nodes per block
XH0 = KT * CI2           # 4160: cols of xh region in strm
SW = XH0 + NB * KT * NSB  # strm width 20544

# qblob column offsets (fp16, [16, QBW])
QO = {"qT": 0, "bru": 512, "bc": 640, "w128ru": 704, "w128c": 832,
      "x64rep": 896}
QBW = 1408

_CACHE = {}


def build_nc(debug=False):
    import concourse.bass as bass
    import concourse.bacc as bacc
    import concourse.tile as tile
    import concourse.mybir as mybir

    F32 = mybir.dt.float32
    F16 = mybir.dt.float16
    ACT = mybir.ActivationFunctionType

    nc = bacc.Bacc()
    dp = nc.declare_dram_parameter
    qblob_e = dp("qblob", [QD, QBW], F16, isOutput=False)
    qbp_e = dp("qbp", [128, NB * QD * NSB], F16, isOutput=False)
    xhblob_e = dp("xhblob", [IN, 2 * NS], F16, isOutput=False)
    strm_e = dp("strm", [128, SW], F16, isOutput=False)
    wdru_e = dp("wdru", [128, QD * 2 * IN], F16, isOutput=False)
    wdc_e = dp("wdc", [128, QD * IN], F16, isOutput=False)
    out_e = dp("out", [IN, NS], F16, isOutput=True)

    def bs(b):
        return slice(b * NSB, (b + 1) * NSB)

    with tile.TileContext(nc) as tc:
        with tc.tile_pool(name="const", bufs=1) as cpool, \
             tc.tile_pool(name="big", bufs=1) as bigpool, \
             tc.tile_pool(name="work", bufs=1) as wpool, \
             tc.tile_pool(name="zt", bufs=3) as ztpool, \
             tc.tile_pool(name="psM", bufs=1, space="PSUM") as psM:

            # ---- DMAs: all consumption-ordered on the sync queue; xhblob
            # rides the scalar queue (only needed by ~19us) -------------------
            qblob = cpool.tile([QD, QBW], F16, tag="qblob")
            nc.sync.dma_start(qblob[:], qblob_e[:])
            xhblob = cpool.tile([IN, 2 * NS], F16, tag="xhblob")
            nc.scalar.dma_start(xhblob[:], xhblob_e[:])

            strm = bigpool.tile([128, SW], F16)
            qbp = cpool.tile([128, NB * QD * NSB], F16, tag="qbp")
            wdru = cpool.tile([128, QD * 2 * IN], F16, tag="wdru")
            wdc = cpool.tile([128, QD * IN], F16, tag="wdc")
            QH = QD * NSB // 2      # 2048 cols: 8 d-tiles of qbp per chunk
            CH = KT * NSB // 2      # 4096 cols = 16 k-tiles per chunk
            nc.sync.dma_start(strm[:, 0:XH0], strm_e[:, 0:XH0])
            for b in range(NB):
                base = XH0 + b * KT * NSB
                nc.sync.dma_start(strm[:, base:base + CH],
                                  strm_e[:, base:base + CH])
                nc.sync.dma_start(strm[:, base + CH:base + 2 * CH],
                                  strm_e[:, base + CH:base + 2 * CH])
                qb0 = b * QD * NSB
                nc.sync.dma_start(qbp[:, qb0:qb0 + QH],
                                  qbp_e[:, qb0:qb0 + QH])
                if b == 0:
                    nc.sync.dma_start(wdru[:], wdru_e[:])
                nc.sync.dma_start(qbp[:, qb0 + QH:qb0 + 2 * QH],
                                  qbp_e[:, qb0 + QH:qb0 + 2 * QH])
                if b == 0:
                    nc.sync.dma_start(wdc[:], wdc_e[:])

            def qslice(name, w):
                o = QO[name]
                return qblob[0:QD, o:o + w]

            qT = qslice("qT", NS)
            xT = xhblob[0:IN, 0:NS]
            hT = xhblob[0:IN, NS:2 * NS]

            def xh_t(t, a, b2):
                return strm[:, t * CI2 + a: t * CI2 + b2]

            def adj_bt(b, t):
                base = XH0 + b * KT * NSB + t * NSB
                return strm[:, base:base + NSB]

            # sel17b: rows {0,32,64,96} = 1 (pl quad-row reduction matrix)
            sel17b = cpool.tile([128, QD], F16, tag="sel17b")
            nc.vector.memset(sel17b[:], 0.0)
            for j in range(4):
                nc.vector.memset(sel17b[32 * j:32 * j + 1, :], 1.0)

            # ---- warmup: dummy matmuls on scrap keep/get the PE hot --------
            scrap = cpool.tile([128, NS], F16, tag="scrap")
            nc.vector.memset(scrap[:], 0.0)
            ps_scrap = psM.tile([QD, NS], F32, tag="scrapps")

            def warm(n):
                for _ in range(n):
                    nc.tensor.matmul(ps_scrap[:], scrap[0:QD, 0:QD],
                                     scrap[0:QD, :], start=True, stop=True,
                                     skip_group_check=True)

            warm(6)

            # preload sigmoid/tanh activation tables off the critical path
            scr_act = wpool.tile([1, 2], F16, tag="scr_act")
            nc.scalar.activation(scr_act[:], scrap[0:1, 0:2], ACT.Sigmoid)
            scr_act2 = wpool.tile([1, 2], F16, tag="scr_act2")
            nc.scalar.activation(scr_act2[:], scrap[0:1, 0:2], ACT.Tanh)

            # xh2T = [h2|x]: x copied into rows 64:128 on ScalarE
            xh2T = wpool.tile([128, NS], F16, tag="xh2T")
            nc.scalar.copy(xh2T[64:128, :], xT)
            # s_c = q (.) x64, needs only inputs
            s_c = wpool.tile([QD, NS], F16, tag="s_c")
            nc.vector.tensor_mul(s_c[:], qT, qslice("x64rep", NS))

            def qb_off(b, d):
                return (b * QD + d) * NSB

            # ---- psum accumulators ----------------------------------------
            gru = psM.tile([128, NS], F32, tag="gru")
            gc = psM.tile([IN, NS], F32, tag="gc")
            pc = psM.tile([128, NS], F32, tag="pc")
            pl = psM.tile([128, NS], F32, tag="pl")
            # quads write only rows {0,32,64,96}; zero the rest so the
            # pl_sb full-tile copy never reads garbage
            nc.vector.memset(pl[:], 0.0)
            v128 = psM.tile([QD, NS], F32, tag="v128")

            # bias matmuls open the PSUM accumulations (one block at a time:
            # a start=True on a sibling slice of the same PSUM bank clobbers
            # the open group, so open block b's group only after block b-1's
            # group on that bank closed)
            def bias_ru(b):
                nc.tensor.matmul(gru[:, bs(b)], qslice("bru", 2 * IN),
                                 qblob[0:QD, bs(b)],
                                 start=True, stop=False, skip_group_check=True)

            def bias_c(b):
                nc.tensor.matmul(gc[:, bs(b)], qslice("bc", IN),
                                 qblob[0:QD, bs(b)],
                                 start=True, stop=False, skip_group_check=True)
                nc.tensor.matmul(gc[:, bs(b)], qslice("w128c", IN),
                                 s_c[:, bs(b)],
                                 start=False, stop=False, skip_group_check=True)

            bias_ru(0)
            bias_c(0)
            warm(2)

            # work tiles (full width, block slices)
            combT = wpool.tile([128, NS], F16, tag="combT")
            pl_sb = wpool.tile([128, NS], F16, tag="pl_sb")
            s_ru = wpool.tile([QD, NS], F16, tag="s_ru")
            r_sb = wpool.tile([IN, NS], F16, tag="r_sb")
            u_sb = wpool.tile([IN, NS], F16, tag="u_sb")
            up_sb = wpool.tile([IN, NS], F16, tag="up_sb")
            cand = wpool.tile([IN, NS], F16, tag="cand")
            e1 = wpool.tile([IN, NS], F16, tag="e1")
            e2 = wpool.tile([IN, NS], F16, tag="e2")
            outT = wpool.tile([IN, NS], F16, tag="outT")

            def adj_phase(b):
                for g in range(8):
                    for t in range(4 * g, 4 * g + 4):
                        nc.tensor.matmul(pc[:, bs(b)], xh_t(t, 0, 128),
                                         adj_bt(b, t),
                                         start=(t == 0), stop=(t == KT - 1),
                                         skip_group_check=True)
                    for t in range(4 * g, 4 * g + 4):
                        j = t % 4
                        nc.tensor.matmul(pl[32 * j:32 * j + 1, bs(b)],
                                         xh_t(t, 128, 129), adj_bt(b, t),
                                         start=(g == 0), stop=(g == 7),
                                         tile_position=(0, 32 * j),
                                         skip_group_check=True)
                nc.scalar.copy(combT[:, bs(b)], pc[:, bs(b)])
                nc.scalar.copy(pl_sb[:, bs(b)], pl[:, bs(b)])
                nc.tensor.matmul(v128[0:QD, bs(b)], sel17b[:], pl_sb[:, bs(b)],
                                 start=True, stop=True, skip_group_check=True)

            def z_quads(b, V, w, wout, acc, name, stop_last=False):
                """4 z-quads [128,4,NSB] = V (.) qbc, each feeding 4 gate
                matmuls accumulating into acc[:, bs(b)]."""
                V4 = V.unsqueeze(1).broadcast_to([128, 4, NSB])
                for jq in range(4):
                    z4 = ztpool.tile([128, 4 * NSB], F16, tag="z",
                                     name=f"z{name}")
                    o = qb_off(b, 4 * jq)
                    nc.vector.tensor_mul(
                        z4[:].rearrange("p (four n) -> p four n", four=4),
                        V4,
                        qbp[:, o:o + 4 * NSB].rearrange(
                            "p (four n) -> p four n", four=4))
                    for k in range(4):
                        d = 4 * jq + k
                        nc.tensor.matmul(acc[:, bs(b)],
                                         w[:, d * wout:(d + 1) * wout],
                                         z4[:, k * NSB:(k + 1) * NSB],
                                         start=False,
                                         stop=(stop_last and d == QD - 1),
                                         skip_group_check=True)

            def ru_gates(b):
                z_quads(b, combT[:, bs(b)], wdru[:], 2 * IN, gru, f"ru{b}")
                nc.vector.tensor_mul(s_ru[:, bs(b)], qblob[0:QD, bs(b)],
                                     v128[0:QD, bs(b)])
                nc.tensor.matmul(gru[:, bs(b)], qslice("w128ru", 2 * IN),
                                 s_ru[:, bs(b)],
                                 start=False, stop=True, skip_group_check=True)
                nc.scalar.activation(r_sb[:, bs(b)], gru[0:IN, bs(b)],
                                     ACT.Sigmoid)
                nc.scalar.activation(u_sb[:, bs(b)], gru[IN:2 * IN, bs(b)],
                                     ACT.Sigmoid)
                nc.scalar.activation(up_sb[:, bs(b)], gru[IN:2 * IN, bs(b)],
                                     ACT.Sigmoid, scale=-1.0)

            def c_gates(b):
                # h2 = r*h -> xh2T rows 0:64; e2 = (1-u)*h2 early
                nc.vector.tensor_mul(xh2T[0:IN, bs(b)], r_sb[:, bs(b)],
                                     hT[:, bs(b)])
                nc.vector.tensor_mul(e2[:, bs(b)], up_sb[:, bs(b)],
                                     xh2T[0:IN, bs(b)])
                z_quads(b, xh2T[:, bs(b)], wdc[:], IN, gc, f"c{b}",
                        stop_last=True)
                nc.scalar.activation(cand[:, bs(b)], gc[0:IN, bs(b)], ACT.Tanh)

            def out_phase(b):
                nc.vector.tensor_mul(e1[:, bs(b)], u_sb[:, bs(b)],
                                     cand[:, bs(b)])
                nc.vector.tensor_add(outT[:, bs(b)], e1[:, bs(b)],
                                     e2[:, bs(b)])
                nc.sync.dma_start(out_e[:, bs(b)], outT[:, bs(b)])

            # gate-c accumulation closes on its last d-matmul: mark stop by
            # emitting c_gates' last matmul with stop=True -> handled below.
            adj_phase(0)
            ru_gates(0)
            bias_ru(1)          # gru-b0 group closed; open b1's
            adj_phase(1)
            c_gates(0)
            bias_c(1)           # gc-b0 group closed; open b1's
            ru_gates(1)
            out_phase(0)
            c_gates(1)
            out_phase(1)

            if debug:
                dbg_tiles = {"combT": combT, "r_sb": r_sb, "u_sb": u_sb,
                             "cand": cand, "xh2T": xh2T, "s_ru": s_ru,
                             "s_c": s_c, "pl_sb": pl_sb, "qbp": qbp,
                             "e1": e1, "e2": e2}
                for nm, t in dbg_tiles.items():
                    shp = list(t[:].shape)
                    de = dp(f"dbg_{nm}", shp, F16, isOutput=True)
                    nc.sync.dma_start(de[:], t[:])
    nc.compile()
    return nc


def _f16(a):
    return np.ascontiguousarray(np.asarray(a, np.float16))


def prep_in_maps(x, h, query_vectors, adj, nodes_ind, W_r, b_r, W_u, b_u, W_c, b_c):
    x = np.asarray(x, np.float32)
    h = np.asarray(h, np.float32)
    q = np.asarray(query_vectors, np.float32)
    adj = np.asarray(adj, np.float32)
    ni = np.asarray(nodes_ind)
    assert np.array_equal(ni, np.arange(N)), "kernel assumes nodes_ind == arange(N)"

    xh = np.concatenate([x, h, np.zeros((N, 1), np.float32)], axis=-1)  # [N,130]
    xh_kt = xh.reshape(KT, 128, CI2).transpose(1, 0, 2)     # [128, KT, 130]

    Wr = np.asarray(W_r, np.float32)
    Wu = np.asarray(W_u, np.float32)
    Wc = np.asarray(W_c, np.float32)
    wdru = np.concatenate([Wr[:, :128, :], Wu[:, :128, :]], axis=2)  # [16,128,128]
    wdru = _f16(wdru.transpose(1, 0, 2).reshape(128, QD * 2 * IN))
    perm_c = list(range(65, CI)) + list(range(0, 64))                # [h2|x]
    wdc = _f16(Wc[:, perm_c, :].transpose(1, 0, 2).reshape(128, QD * IN))

    in_maps = []
    for c in range(NCORES):
        s = slice(c * NS, (c + 1) * NS)
        qs = q[s].T                                             # [16, 512]

        qblob = np.zeros((QD, QBW), np.float32)

        def put(name, arr):
            o = QO[name]
            qblob[0:arr.shape[0], o:o + arr.shape[1]] = arr

        put("qT", qs)
        put("bru", np.concatenate([np.asarray(b_r, np.float32),
                                   np.asarray(b_u, np.float32)], axis=1))
        put("bc", np.asarray(b_c, np.float32))
        put("w128ru", np.concatenate([Wr[:, 128, :], Wu[:, 128, :]], axis=1))
        put("w128c", Wc[:, 64, :])
        put("x64rep", np.tile(x[s, 64], (QD, 1)))

        xhblob = np.concatenate([x[s, 0:64].T, h[s].T], axis=1)  # [64, 1024]

        adjT_kt = adj[s].T.reshape(KT, 128, NS).transpose(1, 0, 2)  # [128,KT,NS]
        strm = np.concatenate(
            [xh_kt.reshape(128, KT * CI2)]
            + [np.ascontiguousarray(adjT_kt[:, :, b * NSB:(b + 1) * NSB])
               .reshape(128, KT * NSB) for b in range(NB)],
            axis=1)
        strm = _f16(strm)

        # qbp: block-major broadcast of q rows: [(b, d), NSB] on all partitions
        qbp = np.broadcast_to(
            qs.reshape(QD, NB, NSB).transpose(1, 0, 2).reshape(1, -1),
            (128, NB * QD * NSB))

        in_maps.append({
            "qblob": _f16(qblob),
            "qbp": _f16(qbp),
            "xhblob": _f16(xhblob),
            "strm": strm,
            "wdru": wdru, "wdc": wdc,
        })
    return in_maps


def kernel(**inputs):
    from concourse.bass_utils import run_bass_kernel_spmd

    if "nc" not in _CACHE:
        _CACHE["nc"] = build_nc()
    nc = _CACHE["nc"]
    in_maps = prep_in_maps(**inputs)
    res = run_bass_kernel_spmd(nc, in_maps, core_ids=list(range(NCORES)))
    out = np.empty((N, IN), np.float32)
    for c in range(NCORES):
        out[c * NS:(c + 1) * NS, :] = res.results[c]["out"].T.astype(np.float32)
    return out
